# revision 35
# baseline (speedup 1.0000x reference)
"""Trainium2 Bass kernel for nn_Lorec (moe_routing LoRA-with-soft-routing).

Computation (per batch b):
  gate_b = softmax(MLP(LayerNorm(ctr[b])))                    [16]
  A_b[i,r] = sum_r' Wa[r*4096+i, r'] gate_b[r']               [4096,16]
  B_b[r,o] = sum_r' Wb[r*4096+o, r'] gate_b[r']               [16,4096]
  out[b] = (x[b] @ A_b) @ B_b * 2.0                           [2048,4096]

Sharding: data-parallel over bs=8 across 8 NeuronCores (one batch per core).
Gating is replicated on every core (tiny); each core selects its own batch's
gate row via a per-core one-hot input. Adapter weights replicated.

v3 design (profile-driven):
  - PE-cycle diet: mm1 is 4-way col-tiled (4 i-chunks' A columns in the four
    32-col array strips, each MM streaming its own x chunk, partials in one
    PSUM bank consolidated by 3 DVE adds); mm2 is 2-way row-tiled (two
    seq-tiles' xa^T stationaries at row strips 0-15/64-79, B replicated at
    both strips).  This keeps PE work ~3x under the DMA floor even when the
    chip power-throttles the PE to K=4/8 half-clock (observed for ~half the
    kernel with all 8 cores streaming).
  - Ramp diet: the 11 small gating inputs ride in 2 packed DMAs (each
    DMA_DIRECT2D costs ~800ns of issue time on its engine); big loads
    (wap, xb0, wbp, xb1-3, in that order) ride the sync HWDGE ring while
    y stores own the scalar ring.  B-gen processes ob-pairs h-outer so G2
    stationaries load 2x per pair instead of per-MM.  PE program order is
    gating -> A-gen -> mm1(b0) -> B-gen -> mm2(b0) -> b1..b3 so the first
    store leaves ~30us in.
  All heavy traffic bf16; SCALING folded into Wb; host casts y -> f32.
"""

import os
import sys

sys.path.insert(0, "/opt/trn_rl_repo")

import numpy as np
import ml_dtypes

BS = 8
SEQ = 2048
IN = 4096
OUT = 4096
R = 16
CTR_OUT = 256
CTR_HID = 60
FD = 16  # FINAL_DIM
LN_EPS = 1e-5
SCALING = 2.0

P = 128
SB = 512  # s-block width
NSB = SEQ // SB  # 4 s-blocks
NC_I = IN // P  # 32 i-chunks
NOB = OUT // 512  # 8 o-blocks

# packed-constant column offsets (f32 pack)
C_CTR, C_GAM, C_BET = 0, 256, 512
C_W1T, C_B1, C_W2T, C_B2, C_SEL = 768, 888, 889, 905, 906
W32 = 920
# bf16 pack
C_TID, C_GMA, C_GM2, C_ONE = 0, 128, 160, 416
WBF = 544

_COMPILED = None


def build_program():
    import concourse.bass as bass
    import concourse.mybir as mybir
    from concourse import bacc
    from concourse.masks import make_identity
    from concourse.tile import TileContext

    f32 = mybir.dt.float32
    bf16 = mybir.dt.bfloat16
    AX = mybir.AxisListType.X
    ALU = mybir.AluOpType
    ACTF = mybir.ActivationFunctionType

    nc = bacc.Bacc("TRN2", target_bir_lowering=False, debug=False, num_devices=BS)

    xt_d = nc.dram_tensor("xt", [NSB, P, NC_I * SB], bf16, kind="ExternalInput").ap()
    pk32_d = nc.dram_tensor("pk32", [P, W32], f32, kind="ExternalInput").ap()
    pkbf_d = nc.dram_tensor("pkbf", [P, WBF], bf16, kind="ExternalInput").ap()
    wap_d = nc.dram_tensor("wap", [P, 2 * IN], bf16, kind="ExternalInput").ap()
    wbp_d = nc.dram_tensor("wbp", [P, 2 * OUT], bf16, kind="ExternalInput").ap()
    y_d = nc.dram_tensor("y", [SEQ, OUT], bf16, kind="ExternalOutput").ap()

    with TileContext(nc) as tc:
        with (
            tc.tile_pool(name="const", bufs=1) as const,
            tc.tile_pool(name="gp", bufs=1) as gp,
            tc.tile_pool(name="xbpool", bufs=2) as xbpool,
            tc.tile_pool(name="pfpool", bufs=2) as pfpool,
            tc.tile_pool(name="xapool", bufs=3) as xapool,
            tc.tile_pool(name="opool", bufs=4) as opool,
            tc.tile_pool(name="psg_pool", bufs=1, space="PSUM") as psg_pool,
            tc.tile_pool(name="psP_pool", bufs=2, space="PSUM") as psP_pool,
            tc.tile_pool(name="pso_pool", bufs=5, space="PSUM") as pso_pool,
        ):
            ident = const.tile([P, P], f32)
            make_identity(nc, ident)

            # ---- packed small inputs: head of the sync ring (land first; the
            # scalar ring is reserved for y stores so these never queue
            # behind 20 MB of weight/x traffic) ----
            pk32 = gp.tile([P, W32], f32)
            pkbf = gp.tile([P, WBF], bf16)
            nc.sync.dma_start(out=pk32[:], in_=pk32_d[:])
            nc.sync.dma_start(out=pkbf[:], in_=pkbf_d[:])
            ctr = pk32[0:BS, C_CTR : C_CTR + CTR_OUT]
            gam = pk32[0:BS, C_GAM : C_GAM + CTR_OUT]
            bet = pk32[0:BS, C_BET : C_BET + CTR_OUT]
            w1t = pk32[0:P, C_W1T : C_W1T + 2 * CTR_HID]
            b1 = pk32[0:CTR_HID, C_B1 : C_B1 + 1]
            w2t = pk32[0:CTR_HID, C_W2T : C_W2T + FD]
            b2 = pk32[0:FD, C_B2 : C_B2 + 1]
            sel = pk32[0:R, C_SEL : C_SEL + BS]
            tid = pkbf[0:FD, C_TID : C_TID + P]
            gmask = pkbf[0:P, C_GMA : C_GMA + 2 * FD]
            gmask2 = pkbf[0:P, C_GM2 : C_GM2 + 2 * P]
            ones16 = pkbf[0:FD, C_ONE : C_ONE + P]

            # ---- big loads on the sync ring: wap, xb0, wbp, xb1-3 ----
            wap_sb = gp.tile([P, 2 * IN], bf16)
            wbp_sb = gp.tile([P, 2 * OUT], bf16)
            xbs = []
            nc.sync.dma_start(out=wap_sb[:], in_=wap_d[:])
            xb0 = xbpool.tile([P, NC_I * SB], bf16, tag="xb", name="xb0")
            nc.sync.dma_start(out=xb0[:], in_=xt_d[0])
            xbs.append(xb0)
            nc.sync.dma_start(out=wbp_sb[:], in_=wbp_d[:])
            for sb in range(1, NSB):
                xb = xbpool.tile([P, NC_I * SB], bf16, tag="xb", name=f"xb{sb}")
                nc.sync.dma_start(out=xb[:], in_=xt_d[sb])
                xbs.append(xb)

            # ---- LayerNorm on [8, 256] ----
            mean = gp.tile([BS, 1], f32)
            xcen = gp.tile([BS, CTR_OUT], f32)
            sq = gp.tile([BS, CTR_OUT], f32)
            vs = gp.tile([BS, 1], f32)
            std = gp.tile([BS, 1], f32)
            rstd = gp.tile([BS, 1], f32)
            hh = gp.tile([BS, CTR_OUT], f32)
            nc.vector.tensor_reduce(mean[:], ctr, axis=AX, op=ALU.add)
            nc.scalar.mul(mean[:], mean[:], 1.0 / CTR_OUT)
            nc.vector.tensor_scalar_sub(xcen[:], ctr, mean[:])
            nc.vector.tensor_mul(sq[:], xcen[:], xcen[:])
            nc.vector.tensor_reduce(vs[:], sq[:], axis=AX, op=ALU.add)
            eps_t = gp.tile([BS, 1], f32)
            nc.vector.memset(eps_t[:], LN_EPS)
            nc.scalar.activation(std[:], vs[:], ACTF.Sqrt, bias=eps_t[:], scale=1.0 / CTR_OUT)
            nc.vector.reciprocal(rstd[:], std[:])
            nc.vector.tensor_scalar_mul(hh[:], xcen[:], rstd[:])
            nc.vector.tensor_mul(hh[:], hh[:], gam)
            nc.vector.tensor_add(hh[:], hh[:], bet)

            # ---- hT [256->2x128, 8] via PE transpose ----
            hT = gp.tile([P, 2 * BS], f32)
            for h in range(2):
                pt = psg_pool.tile([P, BS], f32, tag="psg_small")
                nc.tensor.transpose(pt[:], hh[:, h * P : (h + 1) * P], ident[0:BS, 0:BS])
                nc.scalar.copy(hT[:, h * BS : (h + 1) * BS], pt[:])

            # ---- h1T = relu(W1 @ h + b1) -> [60, 8] ----
            ph1 = psg_pool.tile([CTR_HID, BS], f32, tag="psg_small")
            for h in range(2):
                nc.tensor.matmul(
                    ph1[:], pk32[0:P, C_W1T + h * CTR_HID : C_W1T + (h + 1) * CTR_HID],
                    hT[:, h * BS : (h + 1) * BS], start=(h == 0), stop=(h == 1),
                )
            h1T = gp.tile([CTR_HID, BS], f32)
            nc.scalar.activation(h1T[:], ph1[:], ACTF.Relu, bias=b1)

            # ---- logitsT = W2 @ h1 + b2 -> [16, 8] ----
            plog = psg_pool.tile([FD, BS], f32, tag="psg_small")
            nc.tensor.matmul(plog[:], w2t, h1T[:], start=True, stop=True)
            logitsT = gp.tile([FD, BS], f32)
            nc.scalar.activation(logitsT[:], plog[:], ACTF.Identity, bias=b2)

            # ---- softmax over FD per batch (logits bounded: skip max-sub) ----
            plg = psg_pool.tile([BS, FD], f32, tag="psg_small")
            nc.tensor.transpose(plg[:], logitsT[:], ident[0:FD, 0:FD])
            lg = gp.tile([BS, FD], f32)
            nc.scalar.copy(lg[:], plg[:])
            ex = gp.tile([BS, FD], f32)
            sm = gp.tile([BS, 1], f32)
            rsm = gp.tile([BS, 1], f32)
            gate = gp.tile([BS, FD], f32)
            nc.scalar.activation(ex[:], lg[:], ACTF.Exp)
            nc.vector.tensor_reduce(sm[:], ex[:], axis=AX, op=ALU.add)
            nc.vector.reciprocal(rsm[:], sm[:])
            nc.vector.tensor_scalar_mul(gate[:], ex[:], rsm[:])

            # ---- gateT [16, 8], select own batch via one-hot rows ----
            pgT = psg_pool.tile([FD, BS], f32, tag="psg_small")
            nc.tensor.transpose(pgT[:], gate[:], ident[0:BS, 0:BS])
            gateT = gp.tile([FD, BS], f32)
            nc.scalar.copy(gateT[:], pgT[:])
            gsel = gp.tile([FD, BS], f32)
            gate_b = gp.tile([FD, 1], f32)
            nc.vector.tensor_mul(gsel[:], gateT[:], sel)
            nc.vector.tensor_reduce(gate_b[:], gsel[:], axis=AX, op=ALU.add)

            # ---- gate_rep[p] = gate_b[p % 16] via one PE broadcast matmul ----
            gate_bb = gp.tile([FD, 1], bf16)
            nc.scalar.copy(gate_bb[:], gate_b[:])
            psG = psg_pool.tile([P, 1], f32, tag="psg_small")
            nc.tensor.matmul(psG[:], tid, gate_bb[:], start=True, stop=True)
            gate_rep = gp.tile([P, 1], f32)
            nc.scalar.copy(gate_rep[:], psG[:])
            # G = I_16 kron gate_b (A-gen), G2 = same replicated at 4 strips
            G = gp.tile([P, 2 * FD], bf16)
            nc.vector.tensor_scalar_mul(G[:], gmask, gate_rep[:])
            G2 = gp.tile([P, 2 * P], bf16)
            nc.vector.tensor_scalar_mul(G2[:], gmask2, gate_rep[:])

            # ---- A-gen: A_sb[p, c*16+r] = A[c*128+p, r] (bf16) ----
            A_sb = gp.tile([P, NC_I * R], bf16)
            psA = psP_pool.tile([P, 512], f32, tag="psP", name="psA")
            for c in range(NC_I):
                for h in range(2):
                    nc.tensor.matmul(
                        psA[:, c * R : (c + 1) * R],
                        wap_sb[:, h * IN + c * P : h * IN + (c + 1) * P],
                        G[:, h * FD : (h + 1) * FD],
                        start=(h == 0), stop=(h == 1),
                    )
            nc.scalar.copy(A_sb[:], psA[:])

            def emit_mm1_slot(sb, q, psP):
                """One 4-way col-tiled mm1 slot (4 concurrent i-chunk MMs)."""
                xb = xbs[sb]
                for j in range(4):
                    c = q * 4 + j
                    nc.tensor.matmul(
                        psP[32 * j : 32 * j + FD, :],
                        A_sb[:, c * R : (c + 1) * R],
                        xb[:, c * SB : (c + 1) * SB],
                        start=(q == 0), stop=(q == NC_I // 4 - 1),
                        tile_position=(0, 32 * j),
                        skip_group_check=True,
                    )

            def emit_mm1(sb):
                """4-way col-tiled mm1 -> partial strips in one PSUM bank."""
                psP = psP_pool.tile([P, SB], f32, tag="psP", name=f"psP{sb}")
                for q in range(NC_I // 4):
                    emit_mm1_slot(sb, q, psP)
                return psP

            def emit_consol(sb, psP):
                """Sum the 4 partial strips -> xa^T bf16 at strips 0/32/64/96."""
                pf = pfpool.tile([FD, SB], f32, tag="pf", name=f"pf{sb}")
                nc.vector.tensor_copy(pf[:], psP[0:FD, :])
                nc.vector.tensor_add(pf[:], pf[:], psP[32 : 32 + FD, :])
                nc.vector.tensor_add(pf[:], pf[:], psP[64 : 64 + FD, :])
                xaT4 = xapool.tile([P, SB], bf16, tag="xaT4", name=f"xaT4_{sb}")
                nc.vector.tensor_add(xaT4[0:FD, :], pf[:], psP[96 : 96 + FD, :])
                nc.scalar.copy(xaT4[32 : 32 + FD, :], xaT4[0:FD, :])
                nc.vector.tensor_copy(xaT4[64 : 64 + FD, :], xaT4[0:FD, :])
                nc.scalar.copy(xaT4[96 : 96 + FD, :], xaT4[0:FD, :])
                return xaT4

            def emit_mm2(sb, xaT4, interleave_sb=None):
                """4-way row-tiled mm2 + PSUM->SBUF copies + y stores.

                Strip u computes seq-tile u; copies alternate scalar/vector.
                The PE idles ~0.7us per ob waiting on copies, so mm1 slots of
                block `interleave_sb` are slotted into that hole."""
                last = sb == NSB - 1
                psP_next = None
                if interleave_sb is not None:
                    psP_next = psP_pool.tile([P, SB], f32, tag="psP", name=f"psP{interleave_sb}")
                out2a = opool.tile([P, 2, OUT], bf16, tag="osb", name=f"o{sb}_0")
                out2b = opool.tile([P, 2, OUT], bf16, tag="osb", name=f"o{sb}_1")
                outs = (out2a, out2b)
                for ob in range(NOB):
                    obs_ = slice(ob * 512, (ob + 1) * 512)
                    pss = []
                    for u in range(4):
                        ps = pso_pool.tile([P, 512], f32, tag="pso", name=f"ps{sb}_{ob}_{u}")
                        nc.tensor.matmul(
                            ps[:],
                            xaT4[32 * u : 32 * u + FD, u * P : (u + 1) * P],
                            B4[32 * u : 32 * u + FD, obs_],
                            start=True, stop=True,
                            tile_position=(32 * u, 0),
                        )
                        pss.append(ps)
                    if psP_next is not None:
                        emit_mm1_slot(interleave_sb, ob, psP_next)
                    nc.scalar.copy(out2a[:, 0, obs_], pss[0][:])
                    nc.vector.tensor_copy(out2a[:, 1, obs_], pss[1][:])
                    nc.scalar.copy(out2b[:, 0, obs_], pss[2][:])
                    nc.vector.tensor_copy(out2b[:, 1, obs_], pss[3][:])
                    if ob == NOB // 2 - 1:
                        # store finished column halves early: smooths the
                        # store stream and shrinks the final drain tail.
                        # out2a rides the sync ring (idle once loads finish),
                        # out2b the scalar ring.
                        for tp, eng in ((0, nc.gpsimd), (1, nc.gpsimd)):
                            r0 = sb * SB + tp * 2 * P
                            eng.dma_start(
                                out=y_d[r0 : r0 + 2 * P, 0 : OUT // 2].rearrange(
                                    "(t p) f -> p t f", p=P
                                ),
                                in_=outs[tp][:, :, 0 : OUT // 2],
                            )
                for tp, eng in ((0, nc.gpsimd), (1, nc.gpsimd)):
                    r0 = sb * SB + tp * 2 * P
                    eng.dma_start(
                        out=y_d[r0 : r0 + 2 * P, OUT // 2 : OUT].rearrange(
                            "(t p) f -> p t f", p=P
                        ),
                        in_=outs[tp][:, :, OUT // 2 : OUT],
                    )
                return psP_next

            # mm1 of block 0 runs before B-gen (xb0 lands before wbp)
            psP0 = emit_mm1(0)
            xaT4_0 = emit_consol(0, psP0)

            # ---- B-gen: B4 has B at row strips 0-15 and 64-79 (bf16) ----
            # ob-pairs, h-outer: G2 stationary loads 2x per pair
            B4 = gp.tile([P, OUT], bf16)
            for pr in range(NOB // 2):
                psBa = psP_pool.tile([P, 512], f32, tag="psP", name=f"psB{pr}a")
                psBb = psP_pool.tile([P, 512], f32, tag="psP", name=f"psB{pr}b")
                for h in range(2):
                    for k, psB in enumerate((psBa, psBb)):
                        ob = 2 * pr + k
                        nc.tensor.matmul(
                            psB[:],
                            G2[:, h * P : (h + 1) * P],
                            wbp_sb[:, h * OUT + ob * 512 : h * OUT + (ob + 1) * 512],
                            start=(h == 0), stop=(h == 1),
                        )
                # full-bank copy: rows between strips are zeros (G2 zero cols)
                nc.scalar.copy(B4[:, (2 * pr) * 512 : (2 * pr + 1) * 512], psBa[:])
                nc.vector.tensor_copy(B4[:, (2 * pr + 1) * 512 : (2 * pr + 2) * 512], psBb[:])

            # software-pipelined two blocks deep: mm1(1) runs before mm2(0),
            # and mm1(k+2)'s slots ride inside mm2(k)'s copy-paced ob loop so
            # the PE never serializes mm1 against the store pipeline
            psP1 = emit_mm1(1)
            xaT4s = {0: xaT4_0, 1: emit_consol(1, psP1)}
            for k in range(NSB):
                nxt = k + 2 if k + 2 < NSB else None
                psP_next = emit_mm2(k, xaT4s.pop(k), interleave_sb=nxt)
                if nxt is not None:
                    xaT4s[nxt] = emit_consol(nxt, psP_next)

    nc.compile()
    return nc


def host_prep(inputs):
    """Build per-core and shared input arrays from the full problem inputs."""
    bf16 = ml_dtypes.bfloat16
    x = np.asarray(inputs["x"], np.float32)
    pk32 = np.zeros((P, W32), np.float32)
    pk32[0:BS, C_CTR : C_CTR + CTR_OUT] = np.asarray(inputs["ctr_hidden_states"], np.float32)
    pk32[0:BS, C_GAM : C_GAM + CTR_OUT] = np.asarray(inputs["ln_gamma"], np.float32)[None, :]
    pk32[0:BS, C_BET : C_BET + CTR_OUT] = np.asarray(inputs["ln_beta"], np.float32)[None, :]
    W1 = np.asarray(inputs["W1"], np.float32)
    pk32[0:P, C_W1T : C_W1T + 2 * CTR_HID] = (
        W1.T.reshape(2, P, CTR_HID).transpose(1, 0, 2).reshape(P, 2 * CTR_HID)
    )
    pk32[0:CTR_HID, C_B1] = np.asarray(inputs["b1"], np.float32)
    pk32[0:CTR_HID, C_W2T : C_W2T + FD] = np.asarray(inputs["W2"], np.float32).T
    pk32[0:FD, C_B2] = np.asarray(inputs["b2"], np.float32)

    pkbf = np.zeros((P, WBF), np.float32)
    pkbf[0:FD, C_TID : C_TID + P] = np.tile(np.eye(FD, dtype=np.float32), (1, 8))
    pkbf[0:FD, C_ONE : C_ONE + P] = 1.0
    for r in range(FD):
        h = r // 8
        p0 = (r % 8) * 16
        pkbf[p0 : p0 + 16, C_GMA + h * FD + r] = 1.0
        for u in range(4):
            pkbf[p0 : p0 + 16, C_GM2 + h * P + 32 * u + r] = 1.0

    Wa = np.asarray(inputs["Wa"], np.float32)
    WaP = Wa.reshape(R, IN, FD).transpose(0, 2, 1).reshape(R * FD, IN)
    wap = np.ascontiguousarray(
        WaP.reshape(2, P, IN).transpose(1, 0, 2).reshape(P, 2 * IN)
    ).astype(bf16)
    Wb = np.asarray(inputs["Wb"], np.float32) * SCALING
    WbP = Wb.reshape(R, OUT, FD).transpose(0, 2, 1).reshape(R * FD, OUT)
    wbp = np.ascontiguousarray(
        WbP.reshape(2, P, OUT).transpose(1, 0, 2).reshape(P, 2 * OUT)
    ).astype(bf16)

    in_maps = []
    for c in range(BS):
        m = dict(wap=wap, wbp=wbp, pkbf=np.ascontiguousarray(pkbf).astype(bf16))
        pc = pk32.copy()
        pc[0:R, C_SEL + c] = 1.0
        m["pk32"] = np.ascontiguousarray(pc)
        # xt[sb, p, c*512 + s] = x[core][sb*512+s, c*128+p]
        xc = x[c].astype(bf16)
        m["xt"] = np.ascontiguousarray(
            xc.reshape(NSB, SB, NC_I, P).transpose(0, 3, 2, 1).reshape(NSB, P, NC_I * SB)
        )
        in_maps.append(m)
    return in_maps


def get_compiled():
    global _COMPILED
    if _COMPILED is None:
        _COMPILED = build_program()
    return _COMPILED


def run(inputs, trace=False):
    from concourse.bass_utils import run_bass_kernel_spmd

    nc = get_compiled()
    in_maps = host_prep(inputs)
    res = run_bass_kernel_spmd(nc, in_maps, list(range(BS)), trace=trace)
    out = np.stack(
        [np.asarray(res.results[c]["y"], np.float32) for c in range(BS)], axis=0
    )
    return out, res


def kernel(**inputs) -> np.ndarray:
    out, _ = run(inputs, trace=False)
    return out


# revision 36
# speedup vs baseline: 1.0916x; 1.0916x over previous
"""Trainium2 Bass kernel for nn_Lorec (moe_routing LoRA-with-soft-routing).

Computation (per batch b):
  gate_b = softmax(MLP(LayerNorm(ctr[b])))                    [16]
  A_b[i,r] = sum_r' Wa[r*4096+i, r'] gate_b[r']               [4096,16]
  B_b[r,o] = sum_r' Wb[r*4096+o, r'] gate_b[r']               [16,4096]
  out[b] = (x[b] @ A_b) @ B_b * 2.0                           [2048,4096]

Sharding: data-parallel over bs=8 across 8 NeuronCores (one batch per core).
Gating is replicated on every core (tiny); each core selects its own batch's
gate row via a per-core one-hot input. Adapter weights replicated.

v3 design (profile-driven):
  - PE-cycle diet: mm1 is 4-way col-tiled (4 i-chunks' A columns in the four
    32-col array strips, each MM streaming its own x chunk, partials in one
    PSUM bank consolidated by 3 DVE adds); mm2 is 2-way row-tiled (two
    seq-tiles' xa^T stationaries at row strips 0-15/64-79, B replicated at
    both strips).  This keeps PE work ~3x under the DMA floor even when the
    chip power-throttles the PE to K=4/8 half-clock (observed for ~half the
    kernel with all 8 cores streaming).
  - Ramp diet: the 11 small gating inputs ride in 2 packed DMAs (each
    DMA_DIRECT2D costs ~800ns of issue time on its engine); big loads
    (wap, xb0, wbp, xb1-3, in that order) ride the sync HWDGE ring while
    y stores own the scalar ring.  B-gen processes ob-pairs h-outer so G2
    stationaries load 2x per pair instead of per-MM.  PE program order is
    gating -> A-gen -> mm1(b0) -> B-gen -> mm2(b0) -> b1..b3 so the first
    store leaves ~30us in.
  All heavy traffic bf16; SCALING folded into Wb; host casts y -> f32.
"""

import os
import sys

sys.path.insert(0, "/opt/trn_rl_repo")

import numpy as np
import ml_dtypes

BS = 8
SEQ = 2048
IN = 4096
OUT = 4096
R = 16
CTR_OUT = 256
CTR_HID = 60
FD = 16  # FINAL_DIM
LN_EPS = 1e-5
SCALING = 2.0

P = 128
SB = 512  # s-block width
NSB = SEQ // SB  # 4 s-blocks
NC_I = IN // P  # 32 i-chunks
NOB = OUT // 512  # 8 o-blocks

# packed-constant column offsets (f32 pack)
C_CTR, C_GAM, C_BET = 0, 256, 512
C_W1T, C_B1, C_W2T, C_B2, C_SEL = 768, 888, 889, 905, 906
W32 = 920
# bf16 pack
C_TID, C_GMA, C_GM2, C_ONE = 0, 128, 160, 416
WBF = 544

_COMPILED = None


def build_program():
    import concourse.bass as bass
    import concourse.mybir as mybir
    from concourse import bacc
    from concourse.masks import make_identity
    from concourse.tile import TileContext

    f32 = mybir.dt.float32
    bf16 = mybir.dt.bfloat16
    AX = mybir.AxisListType.X
    ALU = mybir.AluOpType
    ACTF = mybir.ActivationFunctionType

    nc = bacc.Bacc("TRN2", target_bir_lowering=False, debug=False, num_devices=BS)

    xt_d = nc.dram_tensor("xt", [NSB, P, NC_I * SB], bf16, kind="ExternalInput").ap()
    pk32_d = nc.dram_tensor("pk32", [P, W32], f32, kind="ExternalInput").ap()
    pkbf_d = nc.dram_tensor("pkbf", [P, WBF], bf16, kind="ExternalInput").ap()
    wap_d = nc.dram_tensor("wap", [P, 2 * IN], bf16, kind="ExternalInput").ap()
    wbp_d = nc.dram_tensor("wbp", [P, 2 * OUT], bf16, kind="ExternalInput").ap()
    y_d = nc.dram_tensor("y", [SEQ, OUT], bf16, kind="ExternalOutput").ap()

    with TileContext(nc) as tc:
        with (
            tc.tile_pool(name="const", bufs=1) as const,
            tc.tile_pool(name="gp", bufs=1) as gp,
            tc.tile_pool(name="xbpool", bufs=2) as xbpool,
            tc.tile_pool(name="pfpool", bufs=2) as pfpool,
            tc.tile_pool(name="xapool", bufs=3) as xapool,
            tc.tile_pool(name="opool", bufs=4) as opool,
            tc.tile_pool(name="psg_pool", bufs=1, space="PSUM") as psg_pool,
            tc.tile_pool(name="psP_pool", bufs=2, space="PSUM") as psP_pool,
            tc.tile_pool(name="pso_pool", bufs=4, space="PSUM") as pso_pool,
        ):
            ident = const.tile([P, P], f32)
            make_identity(nc, ident)

            # ---- packed small inputs: head of the sync ring (land first; the
            # scalar ring is reserved for y stores so these never queue
            # behind 20 MB of weight/x traffic) ----
            pk32 = gp.tile([P, W32], f32)
            pkbf = gp.tile([P, WBF], bf16)
            nc.sync.dma_start(out=pk32[:], in_=pk32_d[:])
            nc.sync.dma_start(out=pkbf[:], in_=pkbf_d[:])
            ctr = pk32[0:BS, C_CTR : C_CTR + CTR_OUT]
            gam = pk32[0:BS, C_GAM : C_GAM + CTR_OUT]
            bet = pk32[0:BS, C_BET : C_BET + CTR_OUT]
            w1t = pk32[0:P, C_W1T : C_W1T + 2 * CTR_HID]
            b1 = pk32[0:CTR_HID, C_B1 : C_B1 + 1]
            w2t = pk32[0:CTR_HID, C_W2T : C_W2T + FD]
            b2 = pk32[0:FD, C_B2 : C_B2 + 1]
            sel = pk32[0:R, C_SEL : C_SEL + BS]
            tid = pkbf[0:FD, C_TID : C_TID + P]
            gmask = pkbf[0:P, C_GMA : C_GMA + 2 * FD]
            gmask2 = pkbf[0:P, C_GM2 : C_GM2 + 2 * P]
            ones16 = pkbf[0:FD, C_ONE : C_ONE + P]

            # ---- big loads on the sync ring: wap, xb0, wbp, xb1-3 ----
            wap_sb = gp.tile([P, 2 * IN], bf16)
            wbp_sb = gp.tile([P, 2 * OUT], bf16)
            xbs = []
            nc.sync.dma_start(out=wap_sb[:], in_=wap_d[:])
            xb0 = xbpool.tile([P, NC_I * SB], bf16, tag="xb", name="xb0")
            nc.sync.dma_start(out=xb0[:], in_=xt_d[0])
            xbs.append(xb0)
            nc.sync.dma_start(out=wbp_sb[:], in_=wbp_d[:])
            for sb in range(1, NSB):
                xb = xbpool.tile([P, NC_I * SB], bf16, tag="xb", name=f"xb{sb}")
                nc.sync.dma_start(out=xb[:], in_=xt_d[sb])
                xbs.append(xb)

            # ---- LayerNorm on [8, 256] ----
            mean = gp.tile([BS, 1], f32)
            xcen = gp.tile([BS, CTR_OUT], f32)
            sq = gp.tile([BS, CTR_OUT], f32)
            vs = gp.tile([BS, 1], f32)
            std = gp.tile([BS, 1], f32)
            rstd = gp.tile([BS, 1], f32)
            hh = gp.tile([BS, CTR_OUT], f32)
            nc.vector.tensor_reduce(mean[:], ctr, axis=AX, op=ALU.add)
            nc.scalar.mul(mean[:], mean[:], 1.0 / CTR_OUT)
            nc.vector.tensor_scalar_sub(xcen[:], ctr, mean[:])
            nc.vector.tensor_mul(sq[:], xcen[:], xcen[:])
            nc.vector.tensor_reduce(vs[:], sq[:], axis=AX, op=ALU.add)
            eps_t = gp.tile([BS, 1], f32)
            nc.vector.memset(eps_t[:], LN_EPS)
            nc.scalar.activation(std[:], vs[:], ACTF.Sqrt, bias=eps_t[:], scale=1.0 / CTR_OUT)
            nc.vector.reciprocal(rstd[:], std[:])
            nc.vector.tensor_scalar_mul(hh[:], xcen[:], rstd[:])
            nc.vector.tensor_mul(hh[:], hh[:], gam)
            nc.vector.tensor_add(hh[:], hh[:], bet)

            # ---- hT [256->2x128, 8] via PE transpose ----
            hT = gp.tile([P, 2 * BS], f32)
            for h in range(2):
                pt = psg_pool.tile([P, BS], f32, tag="psg_small")
                nc.tensor.transpose(pt[:], hh[:, h * P : (h + 1) * P], ident[0:BS, 0:BS])
                nc.scalar.copy(hT[:, h * BS : (h + 1) * BS], pt[:])

            # ---- h1T = relu(W1 @ h + b1) -> [60, 8] ----
            ph1 = psg_pool.tile([CTR_HID, BS], f32, tag="psg_small")
            for h in range(2):
                nc.tensor.matmul(
                    ph1[:], pk32[0:P, C_W1T + h * CTR_HID : C_W1T + (h + 1) * CTR_HID],
                    hT[:, h * BS : (h + 1) * BS], start=(h == 0), stop=(h == 1),
                )
            h1T = gp.tile([CTR_HID, BS], f32)
            nc.scalar.activation(h1T[:], ph1[:], ACTF.Relu, bias=b1)

            # ---- logitsT = W2 @ h1 + b2 -> [16, 8] ----
            plog = psg_pool.tile([FD, BS], f32, tag="psg_small")
            nc.tensor.matmul(plog[:], w2t, h1T[:], start=True, stop=True)
            logitsT = gp.tile([FD, BS], f32)
            nc.scalar.activation(logitsT[:], plog[:], ACTF.Identity, bias=b2)

            # ---- softmax over FD per batch (logits bounded: skip max-sub) ----
            plg = psg_pool.tile([BS, FD], f32, tag="psg_small")
            nc.tensor.transpose(plg[:], logitsT[:], ident[0:FD, 0:FD])
            lg = gp.tile([BS, FD], f32)
            nc.scalar.copy(lg[:], plg[:])
            ex = gp.tile([BS, FD], f32)
            sm = gp.tile([BS, 1], f32)
            rsm = gp.tile([BS, 1], f32)
            gate = gp.tile([BS, FD], f32)
            nc.scalar.activation(ex[:], lg[:], ACTF.Exp)
            nc.vector.tensor_reduce(sm[:], ex[:], axis=AX, op=ALU.add)
            nc.vector.reciprocal(rsm[:], sm[:])
            nc.vector.tensor_scalar_mul(gate[:], ex[:], rsm[:])

            # ---- gateT [16, 8], select own batch via one-hot rows ----
            pgT = psg_pool.tile([FD, BS], f32, tag="psg_small")
            nc.tensor.transpose(pgT[:], gate[:], ident[0:BS, 0:BS])
            gateT = gp.tile([FD, BS], f32)
            nc.scalar.copy(gateT[:], pgT[:])
            gsel = gp.tile([FD, BS], f32)
            gate_b = gp.tile([FD, 1], f32)
            nc.vector.tensor_mul(gsel[:], gateT[:], sel)
            nc.vector.tensor_reduce(gate_b[:], gsel[:], axis=AX, op=ALU.add)

            # ---- gate_rep[p] = gate_b[p % 16] via one PE broadcast matmul ----
            gate_bb = gp.tile([FD, 1], bf16)
            nc.scalar.copy(gate_bb[:], gate_b[:])
            psG = psg_pool.tile([P, 1], f32, tag="psg_small")
            nc.tensor.matmul(psG[:], tid, gate_bb[:], start=True, stop=True)
            gate_rep = gp.tile([P, 1], f32)
            nc.scalar.copy(gate_rep[:], psG[:])
            # G = I_16 kron gate_b (A-gen), G2 = same replicated at 4 strips
            G = gp.tile([P, 2 * FD], bf16)
            nc.vector.tensor_scalar_mul(G[:], gmask, gate_rep[:])
            G2 = gp.tile([P, 2 * P], bf16)
            nc.vector.tensor_scalar_mul(G2[:], gmask2, gate_rep[:])

            # ---- A-gen: A_sb[p, c*16+r] = A[c*128+p, r] (bf16) ----
            A_sb = gp.tile([P, NC_I * R], bf16)
            psA = psP_pool.tile([P, 512], f32, tag="psP", name="psA")
            for c in range(NC_I):
                for h in range(2):
                    nc.tensor.matmul(
                        psA[:, c * R : (c + 1) * R],
                        wap_sb[:, h * IN + c * P : h * IN + (c + 1) * P],
                        G[:, h * FD : (h + 1) * FD],
                        start=(h == 0), stop=(h == 1),
                    )
            nc.scalar.copy(A_sb[:], psA[:])

            def emit_mm1_slot(sb, q, psP):
                """One 4-way col-tiled mm1 slot (4 concurrent i-chunk MMs)."""
                xb = xbs[sb]
                for j in range(4):
                    c = q * 4 + j
                    nc.tensor.matmul(
                        psP[32 * j : 32 * j + FD, :],
                        A_sb[:, c * R : (c + 1) * R],
                        xb[:, c * SB : (c + 1) * SB],
                        start=(q == 0), stop=(q == NC_I // 4 - 1),
                        tile_position=(0, 32 * j),
                        skip_group_check=True,
                    )

            def emit_mm1(sb):
                """4-way col-tiled mm1 -> partial strips in one PSUM bank."""
                psP = psP_pool.tile([P, SB], f32, tag="psP", name=f"psP{sb}")
                for q in range(NC_I // 4):
                    emit_mm1_slot(sb, q, psP)
                return psP

            def emit_consol(sb, psP):
                """Sum the 4 partial strips -> xa^T bf16 at strips 0/32/64/96."""
                pf = pfpool.tile([FD, SB], f32, tag="pf", name=f"pf{sb}")
                nc.vector.tensor_copy(pf[:], psP[0:FD, :])
                nc.vector.tensor_add(pf[:], pf[:], psP[32 : 32 + FD, :])
                nc.vector.tensor_add(pf[:], pf[:], psP[64 : 64 + FD, :])
                xaT4 = xapool.tile([P, SB], bf16, tag="xaT4", name=f"xaT4_{sb}")
                nc.vector.tensor_add(xaT4[0:FD, :], pf[:], psP[96 : 96 + FD, :])
                nc.scalar.copy(xaT4[32 : 32 + FD, :], xaT4[0:FD, :])
                nc.vector.tensor_copy(xaT4[64 : 64 + FD, :], xaT4[0:FD, :])
                nc.scalar.copy(xaT4[96 : 96 + FD, :], xaT4[0:FD, :])
                return xaT4

            def emit_mm2(sb, xaT4, interleave_sb=None):
                """4-way row-tiled mm2 + PSUM->SBUF copies + y stores.

                Strip u computes seq-tile u; copies alternate scalar/vector.
                The PE idles ~0.7us per ob waiting on copies, so mm1 slots of
                block `interleave_sb` are slotted into that hole."""
                last = sb == NSB - 1
                psP_next = None
                if interleave_sb is not None:
                    psP_next = psP_pool.tile([P, SB], f32, tag="psP", name=f"psP{interleave_sb}")
                out2a = opool.tile([P, 2, OUT], bf16, tag="osb", name=f"o{sb}_0")
                out2b = opool.tile([P, 2, OUT], bf16, tag="osb", name=f"o{sb}_1")
                outs = (out2a, out2b)
                for ob in range(NOB):
                    obs_ = slice(ob * 512, (ob + 1) * 512)
                    pss = []
                    for u in range(4):
                        ps = pso_pool.tile([P, 512], f32, tag="pso", name=f"ps{sb}_{ob}_{u}")
                        nc.tensor.matmul(
                            ps[:],
                            xaT4[32 * u : 32 * u + FD, u * P : (u + 1) * P],
                            B4[32 * u : 32 * u + FD, obs_],
                            start=True, stop=True,
                            tile_position=(32 * u, 0),
                        )
                        pss.append(ps)
                    if psP_next is not None:
                        emit_mm1_slot(interleave_sb, ob, psP_next)
                    nc.scalar.copy(out2a[:, 0, obs_], pss[0][:])
                    nc.vector.tensor_copy(out2a[:, 1, obs_], pss[1][:])
                    nc.scalar.copy(out2b[:, 0, obs_], pss[2][:])
                    nc.vector.tensor_copy(out2b[:, 1, obs_], pss[3][:])
                    if ob == NOB // 2 - 1:
                        # store finished column halves early: smooths the
                        # store stream and shrinks the final drain tail.
                        # out2a rides the sync ring (idle once loads finish),
                        # out2b the scalar ring.
                        for tp, eng in ((0, nc.gpsimd), (1, nc.gpsimd)):
                            r0 = sb * SB + tp * 2 * P
                            eng.dma_start(
                                out=y_d[r0 : r0 + 2 * P, 0 : OUT // 2].rearrange(
                                    "(t p) f -> p t f", p=P
                                ),
                                in_=outs[tp][:, :, 0 : OUT // 2],
                            )
                for tp, eng in ((0, nc.gpsimd), (1, nc.gpsimd)):
                    r0 = sb * SB + tp * 2 * P
                    eng.dma_start(
                        out=y_d[r0 : r0 + 2 * P, OUT // 2 : OUT].rearrange(
                            "(t p) f -> p t f", p=P
                        ),
                        in_=outs[tp][:, :, OUT // 2 : OUT],
                    )
                return psP_next

            # mm1 of block 0 runs before B-gen (xb0 lands before wbp)
            psP0 = emit_mm1(0)
            xaT4_0 = emit_consol(0, psP0)

            # ---- B-gen: B4 has B at row strips 0-15 and 64-79 (bf16) ----
            # ob-pairs, h-outer: G2 stationary loads 2x per pair
            B4 = gp.tile([P, OUT], bf16)
            for pr in range(NOB // 2):
                psBa = psP_pool.tile([P, 512], f32, tag="psP", name=f"psB{pr}a")
                psBb = psP_pool.tile([P, 512], f32, tag="psP", name=f"psB{pr}b")
                for h in range(2):
                    for k, psB in enumerate((psBa, psBb)):
                        ob = 2 * pr + k
                        nc.tensor.matmul(
                            psB[:],
                            G2[:, h * P : (h + 1) * P],
                            wbp_sb[:, h * OUT + ob * 512 : h * OUT + (ob + 1) * 512],
                            start=(h == 0), stop=(h == 1),
                        )
                # full-bank copy: rows between strips are zeros (G2 zero cols)
                nc.scalar.copy(B4[:, (2 * pr) * 512 : (2 * pr + 1) * 512], psBa[:])
                nc.vector.tensor_copy(B4[:, (2 * pr + 1) * 512 : (2 * pr + 2) * 512], psBb[:])

            # software-pipelined two blocks deep: mm1(1) runs before mm2(0),
            # and mm1(k+2)'s slots ride inside mm2(k)'s copy-paced ob loop so
            # the PE never serializes mm1 against the store pipeline
            psP1 = emit_mm1(1)
            xaT4s = {0: xaT4_0, 1: emit_consol(1, psP1)}
            for k in range(NSB):
                nxt = k + 2 if k + 2 < NSB else None
                psP_next = emit_mm2(k, xaT4s.pop(k), interleave_sb=nxt)
                if nxt is not None:
                    xaT4s[nxt] = emit_consol(nxt, psP_next)

    nc.compile()
    return nc


def host_prep(inputs):
    """Build per-core and shared input arrays from the full problem inputs."""
    bf16 = ml_dtypes.bfloat16
    x = np.asarray(inputs["x"], np.float32)
    pk32 = np.zeros((P, W32), np.float32)
    pk32[0:BS, C_CTR : C_CTR + CTR_OUT] = np.asarray(inputs["ctr_hidden_states"], np.float32)
    pk32[0:BS, C_GAM : C_GAM + CTR_OUT] = np.asarray(inputs["ln_gamma"], np.float32)[None, :]
    pk32[0:BS, C_BET : C_BET + CTR_OUT] = np.asarray(inputs["ln_beta"], np.float32)[None, :]
    W1 = np.asarray(inputs["W1"], np.float32)
    pk32[0:P, C_W1T : C_W1T + 2 * CTR_HID] = (
        W1.T.reshape(2, P, CTR_HID).transpose(1, 0, 2).reshape(P, 2 * CTR_HID)
    )
    pk32[0:CTR_HID, C_B1] = np.asarray(inputs["b1"], np.float32)
    pk32[0:CTR_HID, C_W2T : C_W2T + FD] = np.asarray(inputs["W2"], np.float32).T
    pk32[0:FD, C_B2] = np.asarray(inputs["b2"], np.float32)

    pkbf = np.zeros((P, WBF), np.float32)
    pkbf[0:FD, C_TID : C_TID + P] = np.tile(np.eye(FD, dtype=np.float32), (1, 8))
    pkbf[0:FD, C_ONE : C_ONE + P] = 1.0
    for r in range(FD):
        h = r // 8
        p0 = (r % 8) * 16
        pkbf[p0 : p0 + 16, C_GMA + h * FD + r] = 1.0
        for u in range(4):
            pkbf[p0 : p0 + 16, C_GM2 + h * P + 32 * u + r] = 1.0

    Wa = np.asarray(inputs["Wa"], np.float32)
    WaP = Wa.reshape(R, IN, FD).transpose(0, 2, 1).reshape(R * FD, IN)
    wap = np.ascontiguousarray(
        WaP.reshape(2, P, IN).transpose(1, 0, 2).reshape(P, 2 * IN)
    ).astype(bf16)
    Wb = np.asarray(inputs["Wb"], np.float32) * SCALING
    WbP = Wb.reshape(R, OUT, FD).transpose(0, 2, 1).reshape(R * FD, OUT)
    wbp = np.ascontiguousarray(
        WbP.reshape(2, P, OUT).transpose(1, 0, 2).reshape(P, 2 * OUT)
    ).astype(bf16)

    in_maps = []
    for c in range(BS):
        m = dict(wap=wap, wbp=wbp, pkbf=np.ascontiguousarray(pkbf).astype(bf16))
        pc = pk32.copy()
        pc[0:R, C_SEL + c] = 1.0
        m["pk32"] = np.ascontiguousarray(pc)
        # xt[sb, p, c*512 + s] = x[core][sb*512+s, c*128+p]
        xc = x[c].astype(bf16)
        m["xt"] = np.ascontiguousarray(
            xc.reshape(NSB, SB, NC_I, P).transpose(0, 3, 2, 1).reshape(NSB, P, NC_I * SB)
        )
        in_maps.append(m)
    return in_maps


def get_compiled():
    global _COMPILED
    if _COMPILED is None:
        _COMPILED = build_program()
    return _COMPILED


def run(inputs, trace=False):
    from concourse.bass_utils import run_bass_kernel_spmd

    nc = get_compiled()
    in_maps = host_prep(inputs)
    res = run_bass_kernel_spmd(nc, in_maps, list(range(BS)), trace=trace)
    out = np.stack(
        [np.asarray(res.results[c]["y"], np.float32) for c in range(BS)], axis=0
    )
    return out, res


def kernel(**inputs) -> np.ndarray:
    out, _ = run(inputs, trace=False)
    return out


# revision 37
# speedup vs baseline: 1.0919x; 1.0002x over previous
"""Trainium2 Bass kernel for nn_Lorec (moe_routing LoRA-with-soft-routing).

Computation (per batch b):
  gate_b = softmax(MLP(LayerNorm(ctr[b])))                    [16]
  A_b[i,r] = sum_r' Wa[r*4096+i, r'] gate_b[r']               [4096,16]
  B_b[r,o] = sum_r' Wb[r*4096+o, r'] gate_b[r']               [16,4096]
  out[b] = (x[b] @ A_b) @ B_b * 2.0                           [2048,4096]

Sharding: data-parallel over bs=8 across 8 NeuronCores (one batch per core).
Gating is replicated on every core (tiny); each core selects its own batch's
gate row via a per-core one-hot input. Adapter weights replicated.

Design (profile-driven, ~113us vs the 143us pair-structured baseline):
  - PE-cycle diet: mm1 is 4-way col-tiled (4 i-chunks' A columns in the four
    32-col array strips, each MM streaming its own x chunk, partials in one
    PSUM bank consolidated by 3 DVE adds); mm2 is 4-way row-tiled (four
    seq-tiles' xa^T stationaries at row strips 0/32/64/96, B replicated at
    all four strips by B-gen).  Keeps PE work well under the DMA floor even
    when the chip power-throttles the PE to K=4/8 half-clock (observed for
    ~half the kernel with all 8 cores streaming).
  - DMA plumbing: the 11 small gating inputs ride in 2 packed DMAs at the
    head of the sync HWDGE ring, followed by wap, xb0, wbp, xb1-3 (each xb
    is a host-packed [128, 32*512] block = contiguous 32KB rows); y stores
    are issued from the otherwise-idle GPSIMD engine (SWDGE) so their sem
    waits never block the scalar/vector copy streams, and each 256-row
    group stores its column halves as soon as obs 0-3 finish.
  - Software pipeline: mm1(k+2)'s col-tiled slots ride inside mm2(k)'s
    copy-paced ob loop (the PE idles ~0.7us/ob waiting on PSUM->SBUF
    copies), out2 is 4-deep so copies(k+1) overlap stores(k), and B-gen
    processes ob-pairs h-outer so G2 stationaries load 2x per pair.
  - pso_pool bufs=4 exactly: each mm2 quad's strip-u MM reuses strip-u's
    bank every ob, keeping the quad->copy engine pairing aligned (bufs=5
    measured consistently ~10us slower).
  All heavy traffic bf16; SCALING folded into Wb; host casts y -> f32.
"""

import os
import sys

sys.path.insert(0, "/opt/trn_rl_repo")

import numpy as np
import ml_dtypes

BS = 8
SEQ = 2048
IN = 4096
OUT = 4096
R = 16
CTR_OUT = 256
CTR_HID = 60
FD = 16  # FINAL_DIM
LN_EPS = 1e-5
SCALING = 2.0

P = 128
SB = 512  # s-block width
NSB = SEQ // SB  # 4 s-blocks
NC_I = IN // P  # 32 i-chunks
NOB = OUT // 512  # 8 o-blocks

# packed-constant column offsets (f32 pack)
C_CTR, C_GAM, C_BET = 0, 256, 512
C_W1T, C_B1, C_W2T, C_B2, C_SEL = 768, 888, 889, 905, 906
W32 = 920
# bf16 pack
C_TID, C_GMA, C_GM2, C_ONE = 0, 128, 160, 416
WBF = 544

_COMPILED = None


def build_program():
    import concourse.bass as bass
    import concourse.mybir as mybir
    from concourse import bacc
    from concourse.masks import make_identity
    from concourse.tile import TileContext

    f32 = mybir.dt.float32
    bf16 = mybir.dt.bfloat16
    AX = mybir.AxisListType.X
    ALU = mybir.AluOpType
    ACTF = mybir.ActivationFunctionType

    nc = bacc.Bacc("TRN2", target_bir_lowering=False, debug=False, num_devices=BS)

    xt_d = nc.dram_tensor("xt", [NSB, P, NC_I * SB], bf16, kind="ExternalInput").ap()
    pk32_d = nc.dram_tensor("pk32", [P, W32], f32, kind="ExternalInput").ap()
    pkbf_d = nc.dram_tensor("pkbf", [P, WBF], bf16, kind="ExternalInput").ap()
    wap_d = nc.dram_tensor("wap", [P, 2 * IN], bf16, kind="ExternalInput").ap()
    wbp_d = nc.dram_tensor("wbp", [P, 2 * OUT], bf16, kind="ExternalInput").ap()
    y_d = nc.dram_tensor("y", [SEQ, OUT], bf16, kind="ExternalOutput").ap()

    with TileContext(nc) as tc:
        with (
            tc.tile_pool(name="const", bufs=1) as const,
            tc.tile_pool(name="gp", bufs=1) as gp,
            tc.tile_pool(name="xbpool", bufs=2) as xbpool,
            tc.tile_pool(name="pfpool", bufs=2) as pfpool,
            tc.tile_pool(name="xapool", bufs=3) as xapool,
            tc.tile_pool(name="opool", bufs=4) as opool,
            tc.tile_pool(name="psg_pool", bufs=1, space="PSUM") as psg_pool,
            tc.tile_pool(name="psP_pool", bufs=2, space="PSUM") as psP_pool,
            tc.tile_pool(name="pso_pool", bufs=4, space="PSUM") as pso_pool,
        ):
            ident = const.tile([P, P], f32)
            make_identity(nc, ident)

            # ---- packed small inputs: head of the sync ring (land first; the
            # scalar ring is reserved for y stores so these never queue
            # behind 20 MB of weight/x traffic) ----
            pk32 = gp.tile([P, W32], f32)
            pkbf = gp.tile([P, WBF], bf16)
            nc.sync.dma_start(out=pk32[:], in_=pk32_d[:])
            nc.sync.dma_start(out=pkbf[:], in_=pkbf_d[:])
            ctr = pk32[0:BS, C_CTR : C_CTR + CTR_OUT]
            gam = pk32[0:BS, C_GAM : C_GAM + CTR_OUT]
            bet = pk32[0:BS, C_BET : C_BET + CTR_OUT]
            w1t = pk32[0:P, C_W1T : C_W1T + 2 * CTR_HID]
            b1 = pk32[0:CTR_HID, C_B1 : C_B1 + 1]
            w2t = pk32[0:CTR_HID, C_W2T : C_W2T + FD]
            b2 = pk32[0:FD, C_B2 : C_B2 + 1]
            sel = pk32[0:R, C_SEL : C_SEL + BS]
            tid = pkbf[0:FD, C_TID : C_TID + P]
            gmask = pkbf[0:P, C_GMA : C_GMA + 2 * FD]
            gmask2 = pkbf[0:P, C_GM2 : C_GM2 + 2 * P]
            ones16 = pkbf[0:FD, C_ONE : C_ONE + P]

            # ---- big loads on the sync ring: wap, xb0, wbp, xb1-3 ----
            wap_sb = gp.tile([P, 2 * IN], bf16)
            wbp_sb = gp.tile([P, 2 * OUT], bf16)
            xbs = []
            nc.sync.dma_start(out=wap_sb[:], in_=wap_d[:])
            xb0 = xbpool.tile([P, NC_I * SB], bf16, tag="xb", name="xb0")
            nc.sync.dma_start(out=xb0[:], in_=xt_d[0])
            xbs.append(xb0)
            nc.sync.dma_start(out=wbp_sb[:], in_=wbp_d[:])
            for sb in range(1, NSB):
                xb = xbpool.tile([P, NC_I * SB], bf16, tag="xb", name=f"xb{sb}")
                nc.sync.dma_start(out=xb[:], in_=xt_d[sb])
                xbs.append(xb)

            # ---- LayerNorm on [8, 256] ----
            mean = gp.tile([BS, 1], f32)
            xcen = gp.tile([BS, CTR_OUT], f32)
            sq = gp.tile([BS, CTR_OUT], f32)
            vs = gp.tile([BS, 1], f32)
            std = gp.tile([BS, 1], f32)
            rstd = gp.tile([BS, 1], f32)
            hh = gp.tile([BS, CTR_OUT], f32)
            nc.vector.tensor_reduce(mean[:], ctr, axis=AX, op=ALU.add)
            nc.scalar.mul(mean[:], mean[:], 1.0 / CTR_OUT)
            nc.vector.tensor_scalar_sub(xcen[:], ctr, mean[:])
            nc.vector.tensor_mul(sq[:], xcen[:], xcen[:])
            nc.vector.tensor_reduce(vs[:], sq[:], axis=AX, op=ALU.add)
            eps_t = gp.tile([BS, 1], f32)
            nc.vector.memset(eps_t[:], LN_EPS)
            nc.scalar.activation(std[:], vs[:], ACTF.Sqrt, bias=eps_t[:], scale=1.0 / CTR_OUT)
            nc.vector.reciprocal(rstd[:], std[:])
            nc.vector.tensor_scalar_mul(hh[:], xcen[:], rstd[:])
            nc.vector.tensor_mul(hh[:], hh[:], gam)
            nc.vector.tensor_add(hh[:], hh[:], bet)

            # ---- hT [256->2x128, 8] via PE transpose ----
            hT = gp.tile([P, 2 * BS], f32)
            for h in range(2):
                pt = psg_pool.tile([P, BS], f32, tag="psg_small")
                nc.tensor.transpose(pt[:], hh[:, h * P : (h + 1) * P], ident[0:BS, 0:BS])
                nc.scalar.copy(hT[:, h * BS : (h + 1) * BS], pt[:])

            # ---- h1T = relu(W1 @ h + b1) -> [60, 8] ----
            ph1 = psg_pool.tile([CTR_HID, BS], f32, tag="psg_small")
            for h in range(2):
                nc.tensor.matmul(
                    ph1[:], pk32[0:P, C_W1T + h * CTR_HID : C_W1T + (h + 1) * CTR_HID],
                    hT[:, h * BS : (h + 1) * BS], start=(h == 0), stop=(h == 1),
                )
            h1T = gp.tile([CTR_HID, BS], f32)
            nc.scalar.activation(h1T[:], ph1[:], ACTF.Relu, bias=b1)

            # ---- logitsT = W2 @ h1 + b2 -> [16, 8] ----
            plog = psg_pool.tile([FD, BS], f32, tag="psg_small")
            nc.tensor.matmul(plog[:], w2t, h1T[:], start=True, stop=True)
            logitsT = gp.tile([FD, BS], f32)
            nc.scalar.activation(logitsT[:], plog[:], ACTF.Identity, bias=b2)

            # ---- softmax over FD per batch (logits bounded: skip max-sub) ----
            plg = psg_pool.tile([BS, FD], f32, tag="psg_small")
            nc.tensor.transpose(plg[:], logitsT[:], ident[0:FD, 0:FD])
            lg = gp.tile([BS, FD], f32)
            nc.scalar.copy(lg[:], plg[:])
            ex = gp.tile([BS, FD], f32)
            sm = gp.tile([BS, 1], f32)
            rsm = gp.tile([BS, 1], f32)
            gate = gp.tile([BS, FD], f32)
            nc.scalar.activation(ex[:], lg[:], ACTF.Exp)
            nc.vector.tensor_reduce(sm[:], ex[:], axis=AX, op=ALU.add)
            nc.vector.reciprocal(rsm[:], sm[:])
            nc.vector.tensor_scalar_mul(gate[:], ex[:], rsm[:])

            # ---- gateT [16, 8], select own batch via one-hot rows ----
            pgT = psg_pool.tile([FD, BS], f32, tag="psg_small")
            nc.tensor.transpose(pgT[:], gate[:], ident[0:BS, 0:BS])
            gateT = gp.tile([FD, BS], f32)
            nc.scalar.copy(gateT[:], pgT[:])
            gsel = gp.tile([FD, BS], f32)
            gate_b = gp.tile([FD, 1], f32)
            nc.vector.tensor_mul(gsel[:], gateT[:], sel)
            nc.vector.tensor_reduce(gate_b[:], gsel[:], axis=AX, op=ALU.add)

            # ---- gate_rep[p] = gate_b[p % 16] via one PE broadcast matmul ----
            gate_bb = gp.tile([FD, 1], bf16)
            nc.scalar.copy(gate_bb[:], gate_b[:])
            psG = psg_pool.tile([P, 1], f32, tag="psg_small")
            nc.tensor.matmul(psG[:], tid, gate_bb[:], start=True, stop=True)
            gate_rep = gp.tile([P, 1], f32)
            nc.scalar.copy(gate_rep[:], psG[:])
            # G = I_16 kron gate_b (A-gen), G2 = same replicated at 4 strips
            G = gp.tile([P, 2 * FD], bf16)
            nc.vector.tensor_scalar_mul(G[:], gmask, gate_rep[:])
            G2 = gp.tile([P, 2 * P], bf16)
            nc.vector.tensor_scalar_mul(G2[:], gmask2, gate_rep[:])

            # ---- A-gen: A_sb[p, c*16+r] = A[c*128+p, r] (bf16) ----
            A_sb = gp.tile([P, NC_I * R], bf16)
            psA = psP_pool.tile([P, 512], f32, tag="psP", name="psA")
            for c in range(NC_I):
                for h in range(2):
                    nc.tensor.matmul(
                        psA[:, c * R : (c + 1) * R],
                        wap_sb[:, h * IN + c * P : h * IN + (c + 1) * P],
                        G[:, h * FD : (h + 1) * FD],
                        start=(h == 0), stop=(h == 1),
                    )
            nc.scalar.copy(A_sb[:], psA[:])

            def emit_mm1_slot(sb, q, psP):
                """One 4-way col-tiled mm1 slot (4 concurrent i-chunk MMs)."""
                xb = xbs[sb]
                for j in range(4):
                    c = q * 4 + j
                    nc.tensor.matmul(
                        psP[32 * j : 32 * j + FD, :],
                        A_sb[:, c * R : (c + 1) * R],
                        xb[:, c * SB : (c + 1) * SB],
                        start=(q == 0), stop=(q == NC_I // 4 - 1),
                        tile_position=(0, 32 * j),
                        skip_group_check=True,
                    )

            def emit_mm1(sb):
                """4-way col-tiled mm1 -> partial strips in one PSUM bank."""
                psP = psP_pool.tile([P, SB], f32, tag="psP", name=f"psP{sb}")
                for q in range(NC_I // 4):
                    emit_mm1_slot(sb, q, psP)
                return psP

            def emit_consol(sb, psP):
                """Sum the 4 partial strips -> xa^T bf16 at strips 0/32/64/96."""
                pf = pfpool.tile([FD, SB], f32, tag="pf", name=f"pf{sb}")
                nc.vector.tensor_copy(pf[:], psP[0:FD, :])
                nc.vector.tensor_add(pf[:], pf[:], psP[32 : 32 + FD, :])
                nc.vector.tensor_add(pf[:], pf[:], psP[64 : 64 + FD, :])
                xaT4 = xapool.tile([P, SB], bf16, tag="xaT4", name=f"xaT4_{sb}")
                nc.vector.tensor_add(xaT4[0:FD, :], pf[:], psP[96 : 96 + FD, :])
                nc.scalar.copy(xaT4[32 : 32 + FD, :], xaT4[0:FD, :])
                nc.vector.tensor_copy(xaT4[64 : 64 + FD, :], xaT4[0:FD, :])
                nc.scalar.copy(xaT4[96 : 96 + FD, :], xaT4[0:FD, :])
                return xaT4

            def emit_mm2(sb, xaT4, interleave_sb=None):
                """4-way row-tiled mm2 + PSUM->SBUF copies + y stores.

                Strip u computes seq-tile u; copies alternate scalar/vector.
                The PE idles ~0.7us per ob waiting on copies, so mm1 slots of
                block `interleave_sb` are slotted into that hole."""
                last = sb == NSB - 1
                psP_next = None
                if interleave_sb is not None:
                    psP_next = psP_pool.tile([P, SB], f32, tag="psP", name=f"psP{interleave_sb}")
                out2a = opool.tile([P, 2, OUT], bf16, tag="osb", name=f"o{sb}_0")
                out2b = opool.tile([P, 2, OUT], bf16, tag="osb", name=f"o{sb}_1")
                outs = (out2a, out2b)
                for ob in range(NOB):
                    obs_ = slice(ob * 512, (ob + 1) * 512)
                    pss = []
                    for u in range(4):
                        ps = pso_pool.tile([P, 512], f32, tag="pso", name=f"ps{sb}_{ob}_{u}")
                        nc.tensor.matmul(
                            ps[:],
                            xaT4[32 * u : 32 * u + FD, u * P : (u + 1) * P],
                            B4[32 * u : 32 * u + FD, obs_],
                            start=True, stop=True,
                            tile_position=(32 * u, 0),
                        )
                        pss.append(ps)
                    if psP_next is not None:
                        emit_mm1_slot(interleave_sb, ob, psP_next)
                    nc.scalar.copy(out2a[:, 0, obs_], pss[0][:])
                    nc.vector.tensor_copy(out2a[:, 1, obs_], pss[1][:])
                    nc.scalar.copy(out2b[:, 0, obs_], pss[2][:])
                    nc.vector.tensor_copy(out2b[:, 1, obs_], pss[3][:])
                    if ob == NOB // 2 - 1:
                        # store finished column halves early: smooths the
                        # store stream and shrinks the final drain tail.
                        # out2a rides the sync ring (idle once loads finish),
                        # out2b the scalar ring.
                        for tp, eng in ((0, nc.gpsimd), (1, nc.gpsimd)):
                            r0 = sb * SB + tp * 2 * P
                            eng.dma_start(
                                out=y_d[r0 : r0 + 2 * P, 0 : OUT // 2].rearrange(
                                    "(t p) f -> p t f", p=P
                                ),
                                in_=outs[tp][:, :, 0 : OUT // 2],
                            )
                for tp, eng in ((0, nc.gpsimd), (1, nc.gpsimd)):
                    r0 = sb * SB + tp * 2 * P
                    eng.dma_start(
                        out=y_d[r0 : r0 + 2 * P, OUT // 2 : OUT].rearrange(
                            "(t p) f -> p t f", p=P
                        ),
                        in_=outs[tp][:, :, OUT // 2 : OUT],
                    )
                return psP_next

            # mm1 of block 0 runs before B-gen (xb0 lands before wbp)
            psP0 = emit_mm1(0)
            xaT4_0 = emit_consol(0, psP0)

            # ---- B-gen: B4 has B at row strips 0-15 and 64-79 (bf16) ----
            # ob-pairs, h-outer: G2 stationary loads 2x per pair
            B4 = gp.tile([P, OUT], bf16)
            for pr in range(NOB // 2):
                psBa = psP_pool.tile([P, 512], f32, tag="psP", name=f"psB{pr}a")
                psBb = psP_pool.tile([P, 512], f32, tag="psP", name=f"psB{pr}b")
                for h in range(2):
                    for k, psB in enumerate((psBa, psBb)):
                        ob = 2 * pr + k
                        nc.tensor.matmul(
                            psB[:],
                            G2[:, h * P : (h + 1) * P],
                            wbp_sb[:, h * OUT + ob * 512 : h * OUT + (ob + 1) * 512],
                            start=(h == 0), stop=(h == 1),
                        )
                # full-bank copy: rows between strips are zeros (G2 zero cols)
                nc.scalar.copy(B4[:, (2 * pr) * 512 : (2 * pr + 1) * 512], psBa[:])
                nc.vector.tensor_copy(B4[:, (2 * pr + 1) * 512 : (2 * pr + 2) * 512], psBb[:])

            # software-pipelined two blocks deep: mm1(1) runs before mm2(0),
            # and mm1(k+2)'s slots ride inside mm2(k)'s copy-paced ob loop so
            # the PE never serializes mm1 against the store pipeline
            psP1 = emit_mm1(1)
            xaT4s = {0: xaT4_0, 1: emit_consol(1, psP1)}
            for k in range(NSB):
                nxt = k + 2 if k + 2 < NSB else None
                psP_next = emit_mm2(k, xaT4s.pop(k), interleave_sb=nxt)
                if nxt is not None:
                    xaT4s[nxt] = emit_consol(nxt, psP_next)

    nc.compile()
    return nc


def host_prep(inputs):
    """Build per-core and shared input arrays from the full problem inputs."""
    bf16 = ml_dtypes.bfloat16
    x = np.asarray(inputs["x"], np.float32)
    pk32 = np.zeros((P, W32), np.float32)
    pk32[0:BS, C_CTR : C_CTR + CTR_OUT] = np.asarray(inputs["ctr_hidden_states"], np.float32)
    pk32[0:BS, C_GAM : C_GAM + CTR_OUT] = np.asarray(inputs["ln_gamma"], np.float32)[None, :]
    pk32[0:BS, C_BET : C_BET + CTR_OUT] = np.asarray(inputs["ln_beta"], np.float32)[None, :]
    W1 = np.asarray(inputs["W1"], np.float32)
    pk32[0:P, C_W1T : C_W1T + 2 * CTR_HID] = (
        W1.T.reshape(2, P, CTR_HID).transpose(1, 0, 2).reshape(P, 2 * CTR_HID)
    )
    pk32[0:CTR_HID, C_B1] = np.asarray(inputs["b1"], np.float32)
    pk32[0:CTR_HID, C_W2T : C_W2T + FD] = np.asarray(inputs["W2"], np.float32).T
    pk32[0:FD, C_B2] = np.asarray(inputs["b2"], np.float32)

    pkbf = np.zeros((P, WBF), np.float32)
    pkbf[0:FD, C_TID : C_TID + P] = np.tile(np.eye(FD, dtype=np.float32), (1, 8))
    pkbf[0:FD, C_ONE : C_ONE + P] = 1.0
    for r in range(FD):
        h = r // 8
        p0 = (r % 8) * 16
        pkbf[p0 : p0 + 16, C_GMA + h * FD + r] = 1.0
        for u in range(4):
            pkbf[p0 : p0 + 16, C_GM2 + h * P + 32 * u + r] = 1.0

    Wa = np.asarray(inputs["Wa"], np.float32)
    WaP = Wa.reshape(R, IN, FD).transpose(0, 2, 1).reshape(R * FD, IN)
    wap = np.ascontiguousarray(
        WaP.reshape(2, P, IN).transpose(1, 0, 2).reshape(P, 2 * IN)
    ).astype(bf16)
    Wb = np.asarray(inputs["Wb"], np.float32) * SCALING
    WbP = Wb.reshape(R, OUT, FD).transpose(0, 2, 1).reshape(R * FD, OUT)
    wbp = np.ascontiguousarray(
        WbP.reshape(2, P, OUT).transpose(1, 0, 2).reshape(P, 2 * OUT)
    ).astype(bf16)

    in_maps = []
    for c in range(BS):
        m = dict(wap=wap, wbp=wbp, pkbf=np.ascontiguousarray(pkbf).astype(bf16))
        pc = pk32.copy()
        pc[0:R, C_SEL + c] = 1.0
        m["pk32"] = np.ascontiguousarray(pc)
        # xt[sb, p, c*512 + s] = x[core][sb*512+s, c*128+p]
        xc = x[c].astype(bf16)
        m["xt"] = np.ascontiguousarray(
            xc.reshape(NSB, SB, NC_I, P).transpose(0, 3, 2, 1).reshape(NSB, P, NC_I * SB)
        )
        in_maps.append(m)
    return in_maps


def get_compiled():
    global _COMPILED
    if _COMPILED is None:
        _COMPILED = build_program()
    return _COMPILED


def run(inputs, trace=False):
    from concourse.bass_utils import run_bass_kernel_spmd

    nc = get_compiled()
    in_maps = host_prep(inputs)
    res = run_bass_kernel_spmd(nc, in_maps, list(range(BS)), trace=trace)
    out = np.stack(
        [np.asarray(res.results[c]["y"], np.float32) for c in range(BS)], axis=0
    )
    return out, res


def kernel(**inputs) -> np.ndarray:
    out, _ = run(inputs, trace=False)
    return out
